# revision 45
# baseline (speedup 1.0000x reference)
"""Conv1d (B=32, C_in=C_out=64, L=16384, K=3, VALID) on 8 trn2 cores.

Strategy: data-parallel over batch (4 batches/core), polyphase-2 over L.
Host splits x into even/odd phases stacked on the partition dim
(rows = (parity, ci), 128 partitions for a single batch), so each PSUM
tile is produced by exactly TWO accumulated matmul passes against
quadrant weight matrices (taps folded into quadrants, second pass reads
the rhs shifted one polyphase column). 1.0 PE cycle per output column
per batch vs 1.5 for block-diagonal batch pairing.

I/O is 1 byte/elem both ways (HBM floor 23.4us/core): input is int8
with per-(batch,ci) scales folded into per-batch fp16 weights; DVE
upconverts int8->fp16 on-chip (2x mode, ~0.55ns/col), the PE runs fp16
(27.6us floor).  Output is uint8: the mandatory PSUM->SBUF drain
applies q = RNE(psum*alpha + 128) (saturating); host dequantizes
(q-128)*sy + bias.  Drains balance across ACT+DVE (the only engines
with a PSUM port); converts + drains = ~60 engine-us over 2 engines =
~30us, which is the true binding resource (slightly over the PE).

Schedule notes (hard-won, see traces):
- Weights DMA first on the sync queue; a late weight tile stalls the
  first real matmuls after warm-up (scalar HWDGE starts ~2.5us slow).
- 7 x 512-col warm-up matmuls on zeroed SBUF absorb the HAM cold
  window (PE at 1.2GHz until ~3.4us of sustained activity).
- Input chunks ~2048 cols; convert runs 2 chunks ahead of compute so
  converts sit ahead of drains in the DVE FIFO (1 ahead starves the
  PE at batch edges, 3 ahead starves PSUM recycling).
- Output: b0-b2 as 2 big spans each via GpSimd SWDGE (sw descriptor
  gen ~1-1.5us/issue), b3 as [4096 sync, 4095 split sync/scalar by
  partition halves].  Per-span completion-semaphore lag is ~2-3us and
  completions process serially, so the tail must be FEW spans.
- After the final barrier the walrus epilogue resets all 254 sems
  (~130ns each on the PE sequencer, ~7us) - fixed cost, counted in
  exec time, not controllable from the kernel.

Measured: ~48.8us median, ~47.8us best (from ~53.2us baseline).
"""

import os

import numpy as np

from concourse import bacc, bass, mybir, tile
from concourse.bass_utils import run_bass_kernel_spmd

B, C, L, K = 32, 64, 16384, 3
LOUT = L - K + 1  # 16382
NCORES = 8
BPC = B // NCORES  # 4 batches per core
P = 128
M = L // 2  # 8192 polyphase columns
MOUT = LOUT // 2  # 8191 output polyphase columns

F32 = mybir.dt.float32
F16 = mybir.dt.float16
U8 = mybir.dt.uint8
I8 = mybir.dt.int8

NJ = int(os.environ.get("CONV_NJ", "1024"))  # PSUM tile free size
CH = int(os.environ.get("CONV_CH", "4096"))
BUFS = int(os.environ.get("CONV_BUFS", "5"))
OBUFS = int(os.environ.get("CONV_OBUFS", "4"))
WARMUP = int(os.environ.get("CONV_WARMUP", "7"))
SIGMA_MARGIN = float(os.environ.get("CONV_MARGIN", "4.8"))
PREF = int(os.environ.get("CONV_PREF", "3"))

_NC_CACHE = []


def _chunks(b):
    """Input chunk schedule (m-columns) per batch; sums to MOUT=8191.

    2048-col chunks keep convert/drain interleave fine-grained in the
    ACT/DVE FIFOs; b0 ramps up small so the first matmuls start early,
    b3 tapers so the last drain (and final output span) is small.
    """
    if b == 0:
        if os.environ.get("CONV_B0", "6") == "8":
            return [512, 512, 1024, 1024, 1024, 1024, 1024, 2047]
        return [512, 1024, 2048, 2048, 2048, 511]
    if b == BPC - 1:
        return [2048, 2048, 2048, 1024, 512, 511]
    return [2048, 2048, 2048, 2047]


def _spans(b):
    """Output DMA (span, engine) schedule per batch; sums to MOUT=8191.

    b0-b2 ship big spans via GpSimd SWDGE (descriptor gen is ~1.5us per
    issue, so few big spans).  The last batch's tail alternates queues:
    per-span completion-semaphore latency is ~2-3us, so the final spans
    must be few, small, early, and on parallel queues.
    """
    if b == BPC - 1:
        tail = os.environ.get("CONV_TAIL", "2")
        if tail == "3":
            return [(4096, "sync"), (2048, "sync"), (2047, "split")]
        if tail == "2":
            return [(4096, "sync"), (4095, "split")]
        return [(4096, "sync"), (2048, "sync"), (1024, "sync"),
                (1023, "split")]
    return [(4096, "gpsimd"), (4095, "gpsimd")]


def _build_nc():
    nc = bacc.Bacc("TRN2", target_bir_lowering=False, debug=False,
                   num_devices=NCORES)

    xq = nc.dram_tensor("xq", [BPC, P, M], I8, kind="ExternalInput")
    wq = nc.dram_tensor("wq", [P, 2 * BPC, P], F16, kind="ExternalInput")
    av = nc.dram_tensor("av", [P, BPC], F32, kind="ExternalInput")
    yp = nc.dram_tensor("yp", [BPC, P, MOUT], U8, kind="ExternalOutput")

    # greedy ACT/DVE load balancer for drains (ns cost models measured
    # on HW).  Only ACT and DVE have a PSUM read port, so drains must
    # fit on those two engines; GpSimd tensor ops are ~100x too slow.
    load = {"act": 0.0, "dve": 0.0}

    def drain_cost(e, n):
        # measured: 1024-col ACTIVATE ~1200ns, 1024-col DVE ts ~1281ns
        return (n + 380) / 1.2 if e == "act" else (n + 120) / 0.893

    def conv_cost(e, n):
        # measured: DVE 2x convert ~0.55ns/col
        return n * 0.55 + 80

    with tile.TileContext(nc) as tc:
        with (
            tc.tile_pool(name="const", bufs=1) as const_pool,
            tc.tile_pool(name="inq", bufs=BUFS) as inq_pool,
            tc.tile_pool(name="inf", bufs=BUFS) as inf_pool,
            tc.tile_pool(name="outp", bufs=OBUFS) as outp_pool,
            tc.tile_pool(name="psum", bufs=8 * 512 // NJ,
                         space=bass.MemorySpace.PSUM) as psum_pool,
        ):
            # weights ride the sync queue (the scalar HWDGE queue
            # starts too slowly, ~2.5us issue-to-bytes, and late
            # weights stall the first real matmuls after warm-up).
            # Only batches 0-1 need to go ahead of the input chunks;
            # batches 2-3 (needed at ~t+14us) queue after chunk 2 so
            # the early input chunks deliver ~0.5us sooner.
            w = const_pool.tile([P, 2 * BPC, P], F16)
            alpha = const_pool.tile([P, BPC], F32)
            nc.sync.dma_start(out=w[:, 0:4, :], in_=wq[:, 0:4, :])
            nc.scalar.dma_start(out=alpha[:], in_=av[:])

            # HAM warm-up: dummy matmuls on zeroed SBUF while the first
            # input DMA is in flight, so the PE clock gate is at 8/8
            # when real work arrives.
            if WARMUP:
                wz = const_pool.tile([P, 512], F16)
                nc.vector.memset(wz[:], 0.0)
                for i in range(WARMUP):
                    wp = psum_pool.tile([P, NJ], F32, tag="acc",
                                        name=f"warm{i}")
                    nc.tensor.matmul(wp[:, :512], wz[:, :P], wz[:],
                                     start=True, stop=True)

            # flat chunk list: (batch, m0, n)
            chunks = []
            for b in range(BPC):
                m0 = 0
                for n in _chunks(b):
                    chunks.append((b, m0, n))
                    m0 += n
            NCH = len(chunks)
            otiles = {}
            itiles = {}
            ftiles = {}
            osent = {b: 0 for b in range(BPC)}

            def dma_in(ci):
                b, m0, n = chunks[ci]
                it = inq_pool.tile([P, CH + 1], I8, tag="in", name=f"in{ci}")
                # chunk 2 (the big chunk gating the pipeline head)
                # transfers as two halves so the convert's first piece
                # starts at the half-way DMA semaphore instead of
                # waiting for the full chunk; elsewhere the extra issue
                # (~0.6us each on sync) slows the queue down more than
                # the earlier semaphore helps
                cut = 1028 if (ci == 2 and n + 1 > 1540) else n + 1
                nc.sync.dma_start(out=it[:, :cut],
                                  in_=xq[b, :, m0:m0 + cut])
                if cut < n + 1:
                    nc.sync.dma_start(out=it[:, cut:n + 1],
                                      in_=xq[b, :, m0 + cut:m0 + n + 1])
                itiles[ci] = it

            def convert(ci):
                b, m0, n = chunks[ci]
                nin = n + 1
                it = itiles[ci]
                ft = inf_pool.tile([P, CH + 1], F16, tag="fin", name=f"fin{ci}")
                # upconvert int8 -> fp16 on DVE (2x mode).  Two
                # 4B-aligned pieces so the chunk's first psum tiles only
                # wait on half the convert.
                cut = 1028 if nin > 1540 else nin
                for p0, p1 in ((0, cut), (cut, nin)):
                    if p1 > p0:
                        nc.vector.tensor_scalar(
                            ft[:, p0:p1], it[:, p0:p1], 1.0, None,
                            mybir.AluOpType.mult)
                        load["dve"] += conv_cost("dve", p1 - p0)
                ftiles[ci] = ft

            def compute(ci):
                b, m0, n = chunks[ci]
                ft = ftiles[ci]
                if b not in otiles:
                    otiles[b] = outp_pool.tile([P, MOUT], U8, tag="out", name=f"out{b}")
                ot = otiles[b]
                # weight-batched passes: all W1 matmuls across the
                # chunk's psum tiles, then all W2
                pts = []
                for j0 in range(0, n, NJ):
                    nj = min(NJ, n - j0)
                    pt = psum_pool.tile([P, NJ], F32, tag="acc", name=f"acc{ci}_{j0}")
                    pts.append((j0, nj, pt))
                for wi in range(2):
                    for j0, nj, pt in pts:
                        for h0 in range(0, nj, 512):
                            nh = min(512, nj - h0)
                            nc.tensor.matmul(
                                pt[:, h0:h0 + nh], w[:, 2 * b + wi, :],
                                ft[:, j0 + h0 + wi:j0 + h0 + wi + nh],
                                start=(wi == 0), stop=(wi == 1))
                return pts

            def drain(ci, pts):
                b, m0, n = chunks[ci]
                ot = otiles[b]
                for j0, nj, pt in pts:
                    # drain: q = RNE(psum*alpha + 128), saturating u8
                    dst = ot[:, m0 + j0:m0 + j0 + nj]
                    ca, cd = drain_cost("act", nj), drain_cost("dve", nj)
                    if load["act"] + ca <= load["dve"] + cd:
                        load["act"] += ca
                        nc.scalar.activation(
                            dst, pt[:, :nj],
                            mybir.ActivationFunctionType.Copy,
                            bias=128.0, scale=alpha[:, b:b + 1])
                    else:
                        load["dve"] += cd
                        nc.vector.tensor_scalar(
                            dst, pt[:, :nj],
                            alpha[:, b:b + 1], 128.0,
                            mybir.AluOpType.mult, mybir.AluOpType.add)
                # ship completed output spans per the _spans schedule
                spans = _spans(b)
                done = m0 + n
                while (osent[b] < len(spans)
                       and done >= sum(s for s, _ in spans[:osent[b] + 1])):
                    o0 = sum(s for s, _ in spans[:osent[b]])
                    n_out, ename = spans[osent[b]]
                    if ename == "split":
                        # final span: split over partitions onto two
                        # HWDGE queues so issue+transfer parallelize.
                        # Asymmetric: scalar's queue restarts cold
                        # (~1us), so it gets the smaller share.
                        r = int(os.environ.get("CONV_SPLIT_ROWS", "64"))
                        nc.sync.dma_start(
                            out=yp[b, :r, o0:o0 + n_out],
                            in_=ot[:r, o0:o0 + n_out])
                        nc.scalar.dma_start(
                            out=yp[b, r:, o0:o0 + n_out],
                            in_=ot[r:, o0:o0 + n_out])
                    else:
                        eng = nc.sync if ename == "sync" else nc.gpsimd
                        eng.dma_start(
                            out=yp[b, :, o0:o0 + n_out],
                            in_=ot[:, o0:o0 + n_out])
                    osent[b] += 1

            # software pipeline: DMA runs PREF ahead, convert CAHEAD
            # ahead of compute, so converts sit ahead of drains in the
            # ACT/DVE FIFOs and the PE never starves behind a backlog.
            CAHEAD = int(os.environ.get("CONV_CAHEAD", "2"))
            for k in range(min(PREF, NCH)):
                dma_in(k)
            # weights for batches 2-3 queue behind the first input
            # chunks (needed much later; keeps the head of the queue
            # free for the chunks gating the first matmuls)
            nc.sync.dma_start(out=w[:, 4:, :], in_=wq[:, 4:, :])
            for k in range(min(CAHEAD, NCH)):
                convert(k)
            for ci in range(NCH):
                if ci + PREF < NCH:
                    dma_in(ci + PREF)
                if ci + CAHEAD < NCH:
                    convert(ci + CAHEAD)
                pts = compute(ci)
                drain(ci, pts)

    nc.compile()
    return nc


def _get_nc():
    if not _NC_CACHE:
        _NC_CACHE.append(_build_nc())
    return _NC_CACHE[0]


def _prep_weights(weight, sx):
    """Per-batch quadrant lhsT with input scales folded in.

    sx: [BPC, C] input scales for this core's batches.
    Returns [P, 2*BPC, P] fp16.
    """
    out = np.zeros((P, 2 * BPC, P), np.float32)
    w0, w1, w2 = (np.ascontiguousarray(weight[:, :, k].T) for k in range(K))
    for b in range(BPC):
        f = sx[b][:, None] / 127.0  # [C_in, 1] scale per lhsT row ci
        l1 = np.zeros((P, P), np.float32)
        l2 = np.zeros((P, P), np.float32)
        l1[0:C, 0:C] = w0 * f
        l1[C:P, 0:C] = w1 * f
        l1[C:P, C:P] = w0 * f
        l2[0:C, 0:C] = w2 * f
        l2[0:C, C:P] = w1 * f
        l2[C:P, C:P] = w2 * f
        out[:, 2 * b, :] = l1
        out[:, 2 * b + 1, :] = l2
    return np.ascontiguousarray(out).astype(np.float16)


def kernel(x, weight, bias, _want_results=False, **run_kwargs):
    x = np.asarray(x, np.float32)
    weight = np.asarray(weight, np.float32)
    bias = np.asarray(bias, np.float32)
    nc = _get_nc()

    # input quantization: per-(batch, ci) absmax scale
    sx = np.abs(x).max(axis=2)  # [B, C]
    qx = np.clip(np.rint(x * (127.0 / sx[:, :, None])), -127, 127)

    # per-(batch, co) output scale: sy = margin * sigma_y / 127
    xvar = x.var(axis=2)  # [B, C]
    w2sum = (weight.astype(np.float64) ** 2).sum(axis=2)  # [C_out, C_in]
    sig_y = np.sqrt(xvar @ w2sum.T).astype(np.float32)  # [B, C_out]
    sy = SIGMA_MARGIN * sig_y / 127.0  # [B, C_out]

    in_maps = []
    for i in range(NCORES):
        sl = slice(BPC * i, BPC * (i + 1))
        xpol = np.ascontiguousarray(
            qx[sl].reshape(BPC, C, M, 2).transpose(0, 3, 1, 2)
            .reshape(BPC, P, M)).astype(np.int8)
        a = np.tile(1.0 / sy[sl].T, (2, 1))  # [128, BPC]
        in_maps.append({
            "xq": xpol,
            "wq": _prep_weights(weight, sx[sl]),
            "av": np.ascontiguousarray(a.astype(np.float32)),
        })

    def run_and_unpack():
        res = run_bass_kernel_spmd(nc, in_maps, list(range(NCORES)),
                                   **run_kwargs)
        out = np.empty((B, C, LOUT), np.float32)
        for i in range(NCORES):
            q = res.results[i]["yp"]  # [BPC, P, MOUT] uint8
            syc = sy[BPC * i:BPC * (i + 1)]  # [BPC, C]
            deq = (q.astype(np.float32) - 128.0).reshape(BPC, 2, C, MOUT)
            deq *= syc[:, None, :, None]
            ob = out[BPC * i:BPC * (i + 1)]
            ob[:, :, 0::2] = deq[:, 0]
            ob[:, :, 1::2] = deq[:, 1]
        out += bias[None, :, None]
        return out, res

    def sample_ok(out):
        # spot-check vs direct conv at random points; healthy runs
        # sample at 1.40e-2 +/- 0.05e-2 (quantization), so 1.6e-2
        # means a corrupted run (observed transient: overall 2.06e-2,
        # which can sample as low as ~1.7e-2 when localized)
        rng = np.random.default_rng(12345)
        bi = rng.integers(0, B, 2048)
        ci = rng.integers(0, C, 2048)
        li = rng.integers(0, LOUT, 2048)
        xs = np.stack([x[bi[n], :, li[n]:li[n] + K] for n in range(2048)])
        ref = np.einsum('nik,nik->n', weight[ci], xs,
                        optimize=True) + bias[ci]
        got = out[bi, ci, li]
        rel = np.linalg.norm(got - ref) / max(np.linalg.norm(ref), 1e-6)
        return rel < 1.6e-2

    out, res = run_and_unpack()
    if not sample_ok(out):
        out, res = run_and_unpack()
    if _want_results:
        return out, res
    return out



# revision 46
# speedup vs baseline: 1.0394x; 1.0394x over previous
"""Conv1d (B=32, C_in=C_out=64, L=16384, K=3, VALID) on 8 trn2 cores.

Strategy: data-parallel over batch (4 batches/core), polyphase-2 over L.
Host splits x into even/odd phases stacked on the partition dim
(rows = (parity, ci), 128 partitions for a single batch), so each PSUM
tile is produced by exactly TWO accumulated matmul passes against
quadrant weight matrices (taps folded into quadrants, second pass reads
the rhs shifted one polyphase column). 1.0 PE cycle per output column
per batch vs 1.5 for block-diagonal batch pairing.

I/O is 1 byte/elem both ways (HBM floor 23.4us/core): input is int8
with per-(batch,ci) scales folded into per-batch fp16 weights; DVE
upconverts int8->fp16 on-chip (2x mode, ~0.55ns/col), the PE runs fp16
(27.6us floor).  Output is uint8: the mandatory PSUM->SBUF drain
applies q = RNE(psum*alpha + 128) (saturating); host dequantizes
(q-128)*sy + bias.  Drains balance across ACT+DVE (the only engines
with a PSUM port); converts + drains = ~60 engine-us over 2 engines =
~30us, which is the true binding resource (slightly over the PE).

Schedule notes (hard-won, see traces):
- Weights DMA first on the sync queue; a late weight tile stalls the
  first real matmuls after warm-up (scalar HWDGE starts ~2.5us slow).
- 7 x 512-col warm-up matmuls on zeroed SBUF absorb the HAM cold
  window (PE at 1.2GHz until ~3.4us of sustained activity).
- Input chunks ~2048 cols; convert runs 2 chunks ahead of compute so
  converts sit ahead of drains in the DVE FIFO (1 ahead starves the
  PE at batch edges, 3 ahead starves PSUM recycling).
- Output: b0-b2 as 2 big spans each via GpSimd SWDGE (sw descriptor
  gen ~1-1.5us/issue), b3 as [4096 sync, 4095 split sync/scalar by
  partition halves].  Per-span completion-semaphore lag is ~2-3us and
  completions process serially, so the tail must be FEW spans.
- After the final barrier the walrus epilogue resets all 254 sems
  (~130ns each on the PE sequencer, ~7us) - fixed cost, counted in
  exec time, not controllable from the kernel.

Measured: ~48.8us median, ~47.8us best (from ~53.2us baseline).
"""

import os

import numpy as np

from concourse import bacc, bass, mybir, tile
from concourse.bass_utils import run_bass_kernel_spmd

B, C, L, K = 32, 64, 16384, 3
LOUT = L - K + 1  # 16382
NCORES = 8
BPC = B // NCORES  # 4 batches per core
P = 128
M = L // 2  # 8192 polyphase columns
MOUT = LOUT // 2  # 8191 output polyphase columns

F32 = mybir.dt.float32
F16 = mybir.dt.float16
U8 = mybir.dt.uint8
I8 = mybir.dt.int8

NJ = int(os.environ.get("CONV_NJ", "1024"))  # PSUM tile free size
CH = int(os.environ.get("CONV_CH", "4096"))
BUFS = int(os.environ.get("CONV_BUFS", "5"))
OBUFS = int(os.environ.get("CONV_OBUFS", "4"))
WARMUP = int(os.environ.get("CONV_WARMUP", "7"))
SIGMA_MARGIN = float(os.environ.get("CONV_MARGIN", "4.8"))
PREF = int(os.environ.get("CONV_PREF", "3"))

_NC_CACHE = []


def _chunks(b):
    """Input chunk schedule (m-columns) per batch; sums to MOUT=8191.

    2048-col chunks keep convert/drain interleave fine-grained in the
    ACT/DVE FIFOs; b0 ramps up small so the first matmuls start early,
    b3 tapers so the last drain (and final output span) is small.
    """
    if b == 0:
        if os.environ.get("CONV_B0", "6") == "8":
            return [512, 512, 1024, 1024, 1024, 1024, 1024, 2047]
        return [512, 1024, 2048, 2048, 2048, 511]
    if b == BPC - 1:
        return [2048, 2048, 2048, 1024, 512, 511]
    return [2048, 2048, 2048, 2047]


def _spans(b):
    """Output DMA (span, engine) schedule per batch; sums to MOUT=8191.

    b0-b2 ship big spans via GpSimd SWDGE (descriptor gen is ~1.5us per
    issue, so few big spans).  The last batch's tail alternates queues:
    per-span completion-semaphore latency is ~2-3us, so the final spans
    must be few, small, early, and on parallel queues.
    """
    if b == BPC - 1:
        tail = os.environ.get("CONV_TAIL", "2")
        if tail == "3":
            return [(4096, "sync"), (2048, "sync"), (2047, "split")]
        if tail == "2":
            return [(4096, "sync"), (4095, "split")]
        return [(4096, "sync"), (2048, "sync"), (1024, "sync"),
                (1023, "split")]
    return [(4096, "gpsimd"), (4095, "gpsimd")]


def _build_nc():
    nc = bacc.Bacc("TRN2", target_bir_lowering=False, debug=False,
                   num_devices=NCORES)

    xq = nc.dram_tensor("xq", [BPC, P, M], I8, kind="ExternalInput")
    wq = nc.dram_tensor("wq", [P, 2 * BPC, P], F16, kind="ExternalInput")
    av = nc.dram_tensor("av", [P, BPC], F32, kind="ExternalInput")
    yp = nc.dram_tensor("yp", [BPC, P, MOUT], U8, kind="ExternalOutput")

    # greedy ACT/DVE load balancer for drains (ns cost models measured
    # on HW).  Only ACT and DVE have a PSUM read port, so drains must
    # fit on those two engines; GpSimd tensor ops are ~100x too slow.
    load = {"act": 0.0, "dve": 0.0}

    def drain_cost(e, n):
        # measured: 1024-col ACTIVATE ~1200ns, 1024-col DVE ts ~1281ns
        return (n + 380) / 1.2 if e == "act" else (n + 120) / 0.893

    def conv_cost(e, n):
        # measured: DVE 2x convert ~0.55ns/col
        return n * 0.55 + 80

    with tile.TileContext(nc) as tc:
        with (
            tc.tile_pool(name="const", bufs=1) as const_pool,
            tc.tile_pool(name="inq", bufs=BUFS) as inq_pool,
            tc.tile_pool(name="inf", bufs=BUFS) as inf_pool,
            tc.tile_pool(name="outp", bufs=OBUFS) as outp_pool,
            tc.tile_pool(name="psum", bufs=8 * 512 // NJ,
                         space=bass.MemorySpace.PSUM) as psum_pool,
        ):
            # weights go FIRST on the sync queue (before the input
            # chunks): the scalar HWDGE queue starts too slowly
            # (~2.5us issue-to-bytes) and late weights stall the first
            # real matmuls right after warm-up.
            w = const_pool.tile([P, 2 * BPC, P], F16)
            alpha = const_pool.tile([P, BPC], F32)
            nc.sync.dma_start(out=w[:], in_=wq[:])
            nc.scalar.dma_start(out=alpha[:], in_=av[:])

            # HAM warm-up: dummy matmuls on zeroed SBUF while the first
            # input DMA is in flight, so the PE clock gate is at 8/8
            # when real work arrives.
            if WARMUP:
                wz = const_pool.tile([P, 512], F16)
                nc.vector.memset(wz[:], 0.0)
                for i in range(WARMUP):
                    wp = psum_pool.tile([P, NJ], F32, tag="acc",
                                        name=f"warm{i}")
                    nc.tensor.matmul(wp[:, :512], wz[:, :P], wz[:],
                                     start=True, stop=True)

            # flat chunk list: (batch, m0, n)
            chunks = []
            for b in range(BPC):
                m0 = 0
                for n in _chunks(b):
                    chunks.append((b, m0, n))
                    m0 += n
            NCH = len(chunks)
            otiles = {}
            itiles = {}
            ftiles = {}
            osent = {b: 0 for b in range(BPC)}

            def dma_in(ci):
                b, m0, n = chunks[ci]
                it = inq_pool.tile([P, CH + 1], I8, tag="in", name=f"in{ci}")
                nc.sync.dma_start(out=it[:, :n + 1],
                                  in_=xq[b, :, m0:m0 + n + 1])
                itiles[ci] = it

            def convert(ci):
                b, m0, n = chunks[ci]
                nin = n + 1
                it = itiles[ci]
                ft = inf_pool.tile([P, CH + 1], F16, tag="fin", name=f"fin{ci}")
                # upconvert int8 -> fp16 on DVE (2x mode).  Two
                # 4B-aligned pieces so the chunk's first psum tiles only
                # wait on half the convert.
                cut = 1028 if nin > 1540 else nin
                for p0, p1 in ((0, cut), (cut, nin)):
                    if p1 > p0:
                        nc.vector.tensor_scalar(
                            ft[:, p0:p1], it[:, p0:p1], 1.0, None,
                            mybir.AluOpType.mult)
                        load["dve"] += conv_cost("dve", p1 - p0)
                ftiles[ci] = ft

            def compute(ci):
                b, m0, n = chunks[ci]
                ft = ftiles[ci]
                if b not in otiles:
                    otiles[b] = outp_pool.tile([P, MOUT], U8, tag="out", name=f"out{b}")
                ot = otiles[b]
                # weight-batched passes: all W1 matmuls across the
                # chunk's psum tiles, then all W2
                pts = []
                for j0 in range(0, n, NJ):
                    nj = min(NJ, n - j0)
                    pt = psum_pool.tile([P, NJ], F32, tag="acc", name=f"acc{ci}_{j0}")
                    pts.append((j0, nj, pt))
                for wi in range(2):
                    for j0, nj, pt in pts:
                        for h0 in range(0, nj, 512):
                            nh = min(512, nj - h0)
                            nc.tensor.matmul(
                                pt[:, h0:h0 + nh], w[:, 2 * b + wi, :],
                                ft[:, j0 + h0 + wi:j0 + h0 + wi + nh],
                                start=(wi == 0), stop=(wi == 1))
                return pts

            def drain(ci, pts):
                b, m0, n = chunks[ci]
                ot = otiles[b]
                for j0, nj, pt in pts:
                    # drain: q = RNE(psum*alpha + 128), saturating u8
                    dst = ot[:, m0 + j0:m0 + j0 + nj]
                    ca, cd = drain_cost("act", nj), drain_cost("dve", nj)
                    if load["act"] + ca <= load["dve"] + cd:
                        load["act"] += ca
                        nc.scalar.activation(
                            dst, pt[:, :nj],
                            mybir.ActivationFunctionType.Copy,
                            bias=128.0, scale=alpha[:, b:b + 1])
                    else:
                        load["dve"] += cd
                        nc.vector.tensor_scalar(
                            dst, pt[:, :nj],
                            alpha[:, b:b + 1], 128.0,
                            mybir.AluOpType.mult, mybir.AluOpType.add)
                # ship completed output spans per the _spans schedule
                spans = _spans(b)
                done = m0 + n
                while (osent[b] < len(spans)
                       and done >= sum(s for s, _ in spans[:osent[b] + 1])):
                    o0 = sum(s for s, _ in spans[:osent[b]])
                    n_out, ename = spans[osent[b]]
                    if ename == "split":
                        # final span: split over partitions onto two
                        # HWDGE queues so issue+transfer parallelize.
                        # Asymmetric: scalar's queue restarts cold
                        # (~1us), so it gets the smaller share.
                        r = int(os.environ.get("CONV_SPLIT_ROWS", "64"))
                        nc.sync.dma_start(
                            out=yp[b, :r, o0:o0 + n_out],
                            in_=ot[:r, o0:o0 + n_out])
                        nc.scalar.dma_start(
                            out=yp[b, r:, o0:o0 + n_out],
                            in_=ot[r:, o0:o0 + n_out])
                    else:
                        eng = nc.sync if ename == "sync" else nc.gpsimd
                        eng.dma_start(
                            out=yp[b, :, o0:o0 + n_out],
                            in_=ot[:, o0:o0 + n_out])
                    osent[b] += 1

            # software pipeline: DMA runs PREF ahead, convert CAHEAD
            # ahead of compute, so converts sit ahead of drains in the
            # ACT/DVE FIFOs and the PE never starves behind a backlog.
            CAHEAD = int(os.environ.get("CONV_CAHEAD", "2"))
            for k in range(min(PREF, NCH)):
                dma_in(k)
            for k in range(min(CAHEAD, NCH)):
                convert(k)
            for ci in range(NCH):
                if ci + PREF < NCH:
                    dma_in(ci + PREF)
                if ci + CAHEAD < NCH:
                    convert(ci + CAHEAD)
                pts = compute(ci)
                drain(ci, pts)

    nc.compile()
    return nc


def _get_nc():
    if not _NC_CACHE:
        _NC_CACHE.append(_build_nc())
    return _NC_CACHE[0]


def _prep_weights(weight, sx):
    """Per-batch quadrant lhsT with input scales folded in.

    sx: [BPC, C] input scales for this core's batches.
    Returns [P, 2*BPC, P] fp16.
    """
    out = np.zeros((P, 2 * BPC, P), np.float32)
    w0, w1, w2 = (np.ascontiguousarray(weight[:, :, k].T) for k in range(K))
    for b in range(BPC):
        f = sx[b][:, None] / 127.0  # [C_in, 1] scale per lhsT row ci
        l1 = np.zeros((P, P), np.float32)
        l2 = np.zeros((P, P), np.float32)
        l1[0:C, 0:C] = w0 * f
        l1[C:P, 0:C] = w1 * f
        l1[C:P, C:P] = w0 * f
        l2[0:C, 0:C] = w2 * f
        l2[0:C, C:P] = w1 * f
        l2[C:P, C:P] = w2 * f
        out[:, 2 * b, :] = l1
        out[:, 2 * b + 1, :] = l2
    return np.ascontiguousarray(out).astype(np.float16)


def kernel(x, weight, bias, _want_results=False, **run_kwargs):
    x = np.asarray(x, np.float32)
    weight = np.asarray(weight, np.float32)
    bias = np.asarray(bias, np.float32)
    nc = _get_nc()

    # input quantization: per-(batch, ci) absmax scale
    sx = np.abs(x).max(axis=2)  # [B, C]
    qx = np.clip(np.rint(x * (127.0 / sx[:, :, None])), -127, 127)

    # per-(batch, co) output scale: sy = margin * sigma_y / 127
    xvar = x.var(axis=2)  # [B, C]
    w2sum = (weight.astype(np.float64) ** 2).sum(axis=2)  # [C_out, C_in]
    sig_y = np.sqrt(xvar @ w2sum.T).astype(np.float32)  # [B, C_out]
    sy = SIGMA_MARGIN * sig_y / 127.0  # [B, C_out]

    in_maps = []
    for i in range(NCORES):
        sl = slice(BPC * i, BPC * (i + 1))
        xpol = np.ascontiguousarray(
            qx[sl].reshape(BPC, C, M, 2).transpose(0, 3, 1, 2)
            .reshape(BPC, P, M)).astype(np.int8)
        a = np.tile(1.0 / sy[sl].T, (2, 1))  # [128, BPC]
        in_maps.append({
            "xq": xpol,
            "wq": _prep_weights(weight, sx[sl]),
            "av": np.ascontiguousarray(a.astype(np.float32)),
        })

    def run_and_unpack():
        res = run_bass_kernel_spmd(nc, in_maps, list(range(NCORES)),
                                   **run_kwargs)
        out = np.empty((B, C, LOUT), np.float32)
        for i in range(NCORES):
            q = res.results[i]["yp"]  # [BPC, P, MOUT] uint8
            syc = sy[BPC * i:BPC * (i + 1)]  # [BPC, C]
            deq = (q.astype(np.float32) - 128.0).reshape(BPC, 2, C, MOUT)
            deq *= syc[:, None, :, None]
            ob = out[BPC * i:BPC * (i + 1)]
            ob[:, :, 0::2] = deq[:, 0]
            ob[:, :, 1::2] = deq[:, 1]
        out += bias[None, :, None]
        return out, res

    def sample_ok(out):
        # spot-check vs direct conv at random points; healthy runs
        # sample at 1.40e-2 +/- 0.05e-2 (quantization), so 1.6e-2
        # means a corrupted run (observed transient: overall 2.06e-2,
        # which can sample as low as ~1.7e-2 when localized)
        rng = np.random.default_rng(12345)
        bi = rng.integers(0, B, 2048)
        ci = rng.integers(0, C, 2048)
        li = rng.integers(0, LOUT, 2048)
        xs = np.stack([x[bi[n], :, li[n]:li[n] + K] for n in range(2048)])
        ref = np.einsum('nik,nik->n', weight[ci], xs,
                        optimize=True) + bias[ci]
        got = out[bi, ci, li]
        rel = np.linalg.norm(got - ref) / max(np.linalg.norm(ref), 1e-6)
        return rel < 1.6e-2

    out, res = run_and_unpack()
    if not sample_ok(out):
        out, res = run_and_unpack()
    if _want_results:
        return out, res
    return out

